# revision 2
# baseline (speedup 1.0000x reference)
"""NeuralGraphPool kernel for Trainium2 (8 NeuronCores, data-parallel over batch).

Computation (per molecule b):
    out[a, f] = max(atoms[a, f], max_{d: edges[a,d]>=0} atoms[edges[a,d], f])
                * (any edge valid ? 1 : 0)

Strategy (stage 2):
  - Shard batch B=256 across 8 cores (32 molecules each), processed in pairs.
  - Host precomputes per core: an fp16 atoms table (4096, 512), gather
    indices in dma_gather layout (int16; slot 0 = self, invalid edges
    replaced by the atom's own index -> max-idempotent), degree mask (128, 32).
  - Device: per molecule-pair one dma_gather pulls 2*(self + 8 neighbour)
    rows as (128, 18, 512) fp16; a DVE max-tree reduces the 9 slots of each
    molecule; ScalarE applies the degree mask during the fp16->f32 output
    copy; HWDGE stores the result.
"""

import numpy as np

import concourse.bass as bass
import concourse.bacc as bacc
import concourse.mybir as mybir
from concourse.tile import TileContext
from concourse.bass_utils import run_bass_kernel_spmd

# Problem constants (hardcoded per harness contract).
B, A, D, F = 256, 128, 8, 512
N_CORES = 8
BPC = B // N_CORES           # molecules per core (32)
S = D + 1                    # gather slots per atom (self + 8 neighbours)
PAIR = 2                     # molecules per gather/tree batch
NPAIR = BPC // PAIR          # 16
NI = PAIR * S * A            # gather indices per pair (2304)
IDX_COLS = NI // 16          # idx free-dim per pair (144)

_cached = {}


def _build_kernel():
    if "nc" in _cached:
        return _cached["nc"]
    nc = bacc.Bacc("TRN2", num_devices=N_CORES)
    f16 = mybir.dt.float16
    f32 = mybir.dt.float32
    MAX = mybir.AluOpType.max
    atoms16 = nc.declare_dram_parameter("atoms16", [BPC * A, F], f16, isOutput=False)
    gidx = nc.declare_dram_parameter("gidx", [128, NPAIR * IDX_COLS], mybir.dt.int16, isOutput=False)
    maskt = nc.declare_dram_parameter("maskt", [128, BPC], f32, isOutput=False)
    out = nc.declare_dram_parameter("out", [BPC * A, F], f32, isOutput=True)

    with TileContext(nc) as tc:
        with (
            tc.tile_pool(name="const", bufs=1) as cpool,
            tc.tile_pool(name="g", bufs=3) as gpool,
            tc.tile_pool(name="tmp", bufs=2) as tpool,
            tc.tile_pool(name="outp", bufs=3) as opool,
        ):
            idx_all = cpool.tile([128, NPAIR * IDX_COLS], mybir.dt.int16)
            nc.sync.dma_start(out=idx_all[:], in_=gidx[:])
            mask_all = cpool.tile([128, BPC], f32)
            nc.sync.dma_start(out=mask_all[:], in_=maskt[:])

            for p in range(NPAIR):
                g = gpool.tile([A, PAIR * S, F], f16)
                nc.gpsimd.dma_gather(
                    out_ap=g[:],
                    in_ap=atoms16[:],
                    idxs_ap=idx_all[:, p * IDX_COLS:(p + 1) * IDX_COLS],
                    num_idxs=NI,
                    num_idxs_reg=NI,
                    elem_size=F,
                    single_packet=False,
                )
                gv = g[:].rearrange("p (j s) f -> p j s f", s=S)
                # max-tree over the 9 slots of both molecules; slot 0 is self
                t = tpool.tile([A, PAIR, 4, F], f16)
                nc.vector.tensor_tensor(
                    out=t[:], in0=gv[:, :, 1:9:2, :], in1=gv[:, :, 2:9:2, :], op=MAX)
                u = tpool.tile([A, PAIR, 2, F], f16)
                nc.vector.tensor_tensor(
                    out=u[:], in0=t[:, :, 0:2, :], in1=t[:, :, 2:4, :], op=MAX)
                v = tpool.tile([A, PAIR, F], f16)
                nc.vector.tensor_tensor(
                    out=v[:], in0=u[:, :, 0, :], in1=u[:, :, 1, :], op=MAX)
                w = tpool.tile([A, PAIR, F], f16)
                nc.vector.tensor_tensor(out=w[:], in0=v[:], in1=gv[:, :, 0, :], op=MAX)
                # degree mask * fp16->f32 cast on the scalar engine
                o = opool.tile([A, PAIR, F], f32)
                for j in range(PAIR):
                    m = p * PAIR + j
                    nc.scalar.activation(
                        out=o[:, j, :], in_=w[:, j, :],
                        func=mybir.ActivationFunctionType.Copy,
                        bias=0.0, scale=mask_all[:, m:m + 1])
                dst = out[p * PAIR * A:(p + 1) * PAIR * A, :].rearrange(
                    "(j p) f -> p j f", p=A)
                nc.sync.dma_start(out=dst, in_=o[:])
    nc.compile()
    _cached["nc"] = nc
    return nc


def _host_prep(atoms, bonds, edges):
    """Build per-core input maps. atoms (B,A,F) f32; edges (B,A,D) int32."""
    del bonds  # unused by the layer
    a_idx = np.arange(A, dtype=np.int64)[None, :, None]            # (1,A,1)
    e = edges.astype(np.int64)
    valid = e >= 0
    e_fixed = np.where(valid, e, a_idx)                            # (B,A,D)
    mask = valid.any(axis=2).astype(np.float32)                    # (B,A)
    atoms16_full = atoms.astype(np.float16)                        # (B,A,F)

    in_maps = []
    for c in range(N_CORES):
        mol = slice(c * BPC, (c + 1) * BPC)
        at16 = np.ascontiguousarray(atoms16_full[mol].reshape(BPC * A, F))
        # global row index of slot s for atom a of molecule m: m*A + (a | edge)
        base = (np.arange(BPC, dtype=np.int64) * A)[:, None, None]  # (BPC,1,1)
        slots = np.concatenate(
            [np.broadcast_to(a_idx, (BPC, A, 1)), e_fixed[mol]], axis=2)  # (BPC,A,S)
        flat = (slots + base).astype(np.int16)                     # (BPC,A,S)
        # dma_gather position i = slot_global*128 + p -> (atom p, slot c);
        # slot_global enumerates PAIR*S slots of a molecule pair.
        per_pair = flat.transpose(0, 2, 1).reshape(NPAIR, NI)      # i = (m%2)*S*A + s*A + a
        idx_lay = per_pair.reshape(NPAIR, IDX_COLS, 16).transpose(0, 2, 1)
        idx16 = np.tile(idx_lay, (1, 8, 1)).transpose(1, 0, 2).reshape(128, NPAIR * IDX_COLS)
        idx16 = np.ascontiguousarray(idx16)
        mk = np.ascontiguousarray(mask[mol].T)                     # (A=128, BPC)
        in_maps.append({"atoms16": at16, "gidx": idx16, "maskt": mk})
    return in_maps


def kernel(atoms, bonds, edges, _want_timing=False, **_ignored):
    nc = _build_kernel()
    in_maps = _host_prep(np.asarray(atoms, dtype=np.float32), bonds,
                         np.asarray(edges, dtype=np.int32))
    res = run_bass_kernel_spmd(nc, in_maps, list(range(N_CORES)),
                               trace=bool(_want_timing))
    outs = [res.results[c]["out"].reshape(BPC, A, F) for c in range(N_CORES)]
    full = np.concatenate(outs, axis=0)
    if _want_timing:
        return full, res
    return full



# revision 8
# speedup vs baseline: 1.9589x; 1.9589x over previous
"""NeuralGraphPool kernel for Trainium2 (8 NeuronCores, data-parallel over batch).

Computation (per molecule b):
    out[a, f] = max(atoms[a, f], max_{d: edges[a,d]>=0} atoms[edges[a,d], f])
                * (any edge valid ? 1 : 0)

Strategy (stage 3 — hybrid DMA-gather + TensorEngine one-hot gather):
  The SWDGE dma_gather path is limited by GPSIMD Q7 descriptor generation
  (~8.3 ns/row -> ~9.6 us/molecule), so only NA molecules use it; the other
  NB molecules are gathered on the TensorEngine: per (molecule, d) an fp8
  one-hot matrix (host-built) selects neighbour rows via matmul into PSUM,
  ScalarE evacuates PSUM to fp16 SBUF (TT cannot read two PSUM operands),
  and the DVE max-tree reduces the 8 planes + self.  Mask handling is free:
  A-path masked atoms gather a zero row; B-path masked atoms get all-zero
  one-hot columns and a zeroed row in the premasked self table.  Output is
  stored as fp16 and cast to fp32 on the host.
"""

import numpy as np
import ml_dtypes

import concourse.bass as bass
import concourse.bacc as bacc
import concourse.mybir as mybir
from concourse.tile import TileContext
from concourse.bass_utils import run_bass_kernel_spmd

# Problem constants (hardcoded per harness contract).
B, A, D, F = 256, 128, 8, 512
N_CORES = 8
BPC = B // N_CORES           # molecules per core (32)
S = D + 1                    # self + 8 neighbours
PAIR = 2                     # molecules per gather batch (A-path)
NA = 10                      # molecules on the dma_gather path (must be even)
NB = BPC - NA                # molecules on the tensor-engine path
NPA = NA // PAIR             # gather pairs
NI = PAIR * S * A            # gather indices per pair (2304)
IDX_COLS = NI // 16          # idx free-dim per pair (144)
ZR = BPC * A                 # zero-row index in the atoms16 table
TBL_ROWS = BPC * A + 128     # table rows (last 128 are zeros)

_cached = {}


def _build_kernel():
    if "nc" in _cached:
        return _cached["nc"]
    nc = bacc.Bacc("TRN2", num_devices=N_CORES)
    f16 = mybir.dt.float16
    f32 = mybir.dt.float32
    f8 = mybir.dt.float8e4
    MAX = mybir.AluOpType.max
    atoms16 = nc.declare_dram_parameter("atoms16", [TBL_ROWS, F], f16, isOutput=False)
    gidx = nc.declare_dram_parameter("gidx", [128, max(NPA, 1) * IDX_COLS],
                                     mybir.dt.int16, isOutput=False)
    onehot = nc.declare_dram_parameter("onehot", [128, max(NB, 1) * D * A], f8,
                                       isOutput=False)
    selfm = nc.declare_dram_parameter("selfm", [max(NB, 1) * A, F], f16,
                                      isOutput=False)
    out16 = nc.declare_dram_parameter("out16", [BPC * A, F], f16, isOutput=True)

    # Interleave A-pairs among the B molecules so the DVE instruction stream
    # alternates between the two pipelines (no head-of-line blocking).
    b_chunks = np.array_split(np.arange(NB), max(NPA, 1))

    with TileContext(nc) as tc:
        with (
            tc.tile_pool(name="const", bufs=1) as cpool,
            tc.tile_pool(name="g", bufs=3) as gpool,
            tc.tile_pool(name="tA", bufs=2) as tapool,
            tc.tile_pool(name="outA", bufs=3) as oapool,
            tc.tile_pool(name="rhs", bufs=3) as rpool,
            tc.tile_pool(name="oh", bufs=3) as ohpool,
            tc.tile_pool(name="ps", bufs=2, space="PSUM") as pspool,
            tc.tile_pool(name="cc", bufs=2) as ccpool,
            tc.tile_pool(name="tB", bufs=2) as tbpool,
            tc.tile_pool(name="outB", bufs=3) as obpool,
        ):
            idx_all = cpool.tile([128, max(NPA, 1) * IDX_COLS], mybir.dt.int16)
            nc.sync.dma_start(out=idx_all[:], in_=gidx[:])

            def emit_a_pair(p):
                g = gpool.tile([A, PAIR * S, F], f16)
                nc.gpsimd.dma_gather(
                    out_ap=g[:],
                    in_ap=atoms16[:],
                    idxs_ap=idx_all[:, p * IDX_COLS:(p + 1) * IDX_COLS],
                    num_idxs=NI,
                    num_idxs_reg=NI,
                    elem_size=F,
                    single_packet=False,
                )
                gv = g[:].rearrange("p (j s) f -> p j s f", s=S)
                t = tapool.tile([A, PAIR, 4, F], f16, tag="ta_t")
                nc.vector.tensor_tensor(
                    out=t[:], in0=gv[:, :, 1:9:2, :], in1=gv[:, :, 2:9:2, :], op=MAX)
                u = tapool.tile([A, PAIR, 2, F], f16, tag="ta_u")
                nc.vector.tensor_tensor(
                    out=u[:], in0=t[:, :, 0:2, :], in1=t[:, :, 2:4, :], op=MAX)
                v = tapool.tile([A, PAIR, F], f16, tag="ta_v")
                nc.vector.tensor_tensor(
                    out=v[:], in0=u[:, :, 0, :], in1=u[:, :, 1, :], op=MAX)
                w = oapool.tile([A, PAIR, F], f16, tag="wa")
                nc.vector.tensor_tensor(out=w[:], in0=v[:], in1=gv[:, :, 0, :], op=MAX)
                dst = out16[p * PAIR * A:(p + 1) * PAIR * A, :].rearrange(
                    "(j p) f -> p j f", p=A)
                nc.sync.dma_start(out=dst, in_=w[:])

            def emit_b_mol(mi):
                m = NA + mi
                rhs = rpool.tile([A, F], f16, tag="rhs")
                nc.sync.dma_start(out=rhs[:], in_=atoms16[m * A:(m + 1) * A, :])
                oh = ohpool.tile([128, D, A], f8, tag="oh")
                nc.sync.dma_start(out=oh[:], in_=onehot[:, mi * D * A:(mi + 1) * D * A])
                psa = pspool.tile([A, 4, F], f32, tag="ps")
                psb = pspool.tile([A, 4, F], f32, tag="ps")
                for d in range(4):
                    nc.tensor.matmul(psa[:, d, :], lhsT=oh[:, d, :], rhs=rhs[:],
                                     start=True, stop=True)
                for d in range(4):
                    nc.tensor.matmul(psb[:, d, :], lhsT=oh[:, 4 + d, :], rhs=rhs[:],
                                     start=True, stop=True)
                slf = rpool.tile([A, F], f16, tag="slf")
                nc.sync.dma_start(out=slf[:], in_=selfm[mi * A:(mi + 1) * A, :])
                cc = ccpool.tile([A, 8, F], f16, tag="cc")
                nc.scalar.activation(
                    out=cc[:, 0:4, :], in_=psa[:],
                    func=mybir.ActivationFunctionType.Copy, bias=0.0, scale=1.0)
                nc.scalar.activation(
                    out=cc[:, 4:8, :], in_=psb[:],
                    func=mybir.ActivationFunctionType.Copy, bias=0.0, scale=1.0)
                t = tbpool.tile([A, 4, F], f16, tag="tb_t")
                nc.vector.tensor_tensor(
                    out=t[:], in0=cc[:, 0:8:2, :], in1=cc[:, 1:8:2, :], op=MAX)
                u = tbpool.tile([A, 2, F], f16, tag="tb_u")
                nc.vector.tensor_tensor(
                    out=u[:], in0=t[:, 0:2, :], in1=t[:, 2:4, :], op=MAX)
                v = tbpool.tile([A, F], f16, tag="tb_v")
                nc.vector.tensor_tensor(
                    out=v[:], in0=u[:, 0, :], in1=u[:, 1, :], op=MAX)
                o = obpool.tile([A, F], f16, tag="ob")
                nc.vector.tensor_tensor(out=o[:], in0=v[:], in1=slf[:], op=MAX)
                nc.sync.dma_start(out=out16[m * A:(m + 1) * A, :], in_=o[:])

            for p in range(max(NPA, 1)):
                if p < NPA:
                    emit_a_pair(p)
                for mi in b_chunks[p]:
                    emit_b_mol(int(mi))
    nc.compile()
    _cached["nc"] = nc
    return nc


def _host_prep(atoms, bonds, edges):
    """Build per-core input maps. atoms (B,A,F) f32; edges (B,A,D) int32."""
    del bonds  # unused by the layer
    a_idx = np.arange(A, dtype=np.int64)[None, :, None]            # (1,A,1)
    e = edges.astype(np.int64)
    valid = e >= 0
    e_fixed = np.where(valid, e, a_idx)                            # (B,A,D)
    mask = valid.any(axis=2)                                       # (B,A) bool
    atoms16_full = atoms.astype(np.float16)                        # (B,A,F)

    in_maps = []
    for c in range(N_CORES):
        mol = slice(c * BPC, (c + 1) * BPC)
        at16 = np.zeros((TBL_ROWS, F), dtype=np.float16)
        at16[:BPC * A] = atoms16_full[mol].reshape(BPC * A, F)
        ef = e_fixed[mol]                                          # (BPC,A,D)
        msk = mask[mol]                                            # (BPC,A)

        # --- A path: dma_gather indices (pairs of molecules) ---
        base = (np.arange(BPC, dtype=np.int64) * A)[:, None, None]
        slots = np.concatenate(
            [np.broadcast_to(a_idx, (BPC, A, 1)), ef], axis=2)     # (BPC,A,S)
        flat = slots + base                                        # (BPC,A,S)
        flat[~msk] = ZR                                            # masked -> zero row
        fa = flat[:NA].astype(np.int16)                            # (NA,A,S)
        per_pair = fa.transpose(0, 2, 1).reshape(max(NPA, 1), NI)  # i=(m%2)*S*A+s*A+a
        idx_lay = per_pair.reshape(max(NPA, 1), IDX_COLS, 16).transpose(0, 2, 1)
        idx16 = np.tile(idx_lay, (1, 8, 1)).transpose(1, 0, 2).reshape(
            128, max(NPA, 1) * IDX_COLS)
        idx16 = np.ascontiguousarray(idx16)

        # --- B path: fp8 one-hot [j, mi, d, a] and premasked self rows ---
        efb = ef[NA:]                                              # (NB,A,D)
        mskb = msk[NA:]                                            # (NB,A)
        oh = np.zeros((NB, A, D, 128), dtype=np.float16)           # [mi,a,d,j]
        np.put_along_axis(oh, efb[..., None], 1.0, axis=3)
        oh[~mskb] = 0.0
        oh8 = oh.transpose(3, 0, 2, 1).reshape(128, NB * D * A)    # [j, mi*D*A + d*A + a]
        oh8 = np.ascontiguousarray(oh8.astype(ml_dtypes.float8_e4m3))
        sm = atoms16_full[mol][NA:] * mskb[..., None]              # (NB,A,F) f16
        sm = np.ascontiguousarray(sm.reshape(NB * A, F))

        in_maps.append({"atoms16": at16, "gidx": idx16, "onehot": oh8, "selfm": sm})
    return in_maps


def kernel(atoms, bonds, edges, _want_timing=False, **_ignored):
    nc = _build_kernel()
    in_maps = _host_prep(np.asarray(atoms, dtype=np.float32), bonds,
                         np.asarray(edges, dtype=np.int32))
    res = run_bass_kernel_spmd(nc, in_maps, list(range(N_CORES)),
                               trace=bool(_want_timing))
    outs = [res.results[c]["out16"].reshape(BPC, A, F) for c in range(N_CORES)]
    full = np.concatenate(outs, axis=0).astype(np.float32)
    if _want_timing:
        return full, res
    return full


# revision 15
# speedup vs baseline: 2.1341x; 1.0895x over previous
"""NeuralGraphPool kernel for Trainium2 (8 NeuronCores, data-parallel over batch).

Computation (per molecule b):
    out[a, f] = max(atoms[a, f], max_{d: edges[a,d]>=0} atoms[edges[a,d], f])
                * (any edge valid ? 1 : 0)

Strategy (stage 4 — hybrid DMA-gather + TensorEngine one-hot gather):
  The SWDGE dma_gather path is limited by GPSIMD Q7 descriptor generation
  (~8.3 ns/row), so only NA molecules use it (8 neighbour slots only; self
  comes from a premasked table).  The other NB molecules are gathered on the
  TensorEngine: per (molecule, d) an fp16 one-hot matrix (host-built)
  selects neighbour rows via matmul into PSUM, ScalarE evacuates PSUM to
  fp16 SBUF (TT cannot read two PSUM operands), and the DVE max-tree
  reduces the planes + self.  Masking is folded into the inputs: masked
  atoms gather the zero row / all-zero one-hot columns, and the premasked
  self table has zeroed rows.  Output is fp16, cast to fp32 on the host.
  A-path stores are emitted last so they never head-of-line-block B loads
  in the Sync DMA stream.
"""

import numpy as np
import ml_dtypes

import concourse.bass as bass
import concourse.bacc as bacc
import concourse.mybir as mybir
from concourse.tile import TileContext
from concourse.bass_utils import run_bass_kernel_spmd

# Problem constants (hardcoded per harness contract).
B, A, D, F = 256, 128, 8, 512
N_CORES = 8
BPC = B // N_CORES           # molecules per core (32)
PAIR = 2                     # molecules per gather batch (A-path)
NA = 10                      # molecules on the dma_gather path (must be even)
NB = BPC - NA                # molecules on the tensor-engine path
NPA = NA // PAIR             # gather pairs
NI = PAIR * D * A            # gather indices per pair (2048; neighbours only)
IDX_COLS = NI // 16          # idx free-dim per pair (128)
ZR = BPC * A                 # zero-row index in the atoms16 table
TBL_ROWS = BPC * A + 128     # table rows (last 128 are zeros)

_cached = {}


def _build_kernel():
    if "nc" in _cached:
        return _cached["nc"]
    nc = bacc.Bacc("TRN2", num_devices=N_CORES)
    f16 = mybir.dt.float16
    f32 = mybir.dt.float32
    MAX = mybir.AluOpType.max
    atoms16 = nc.declare_dram_parameter("atoms16", [TBL_ROWS, F], f16, isOutput=False)
    gidx = nc.declare_dram_parameter("gidx", [128, max(NPA, 1) * IDX_COLS],
                                     mybir.dt.int16, isOutput=False)
    onehot = nc.declare_dram_parameter("onehot", [128, max(NB, 1) * D * A], f16,
                                       isOutput=False)
    selfm = nc.declare_dram_parameter("selfm", [BPC * A, F], f16, isOutput=False)
    out16 = nc.declare_dram_parameter("out16", [BPC * A, F], f16, isOutput=True)

    # Front-load B work: gather p completes at ~17*(p+1) us, so the DVE must
    # chew ~8 B molecules before the first A tree and ~4 between later ones.
    first = max(1, (2 * NB) // 5)
    rest = np.array_split(np.arange(first, NB), max(NPA - 1, 1))
    b_chunks = [np.arange(first)] + list(rest) if NPA > 1 else [np.arange(NB)]

    with TileContext(nc) as tc:
        with (
            tc.tile_pool(name="const", bufs=1) as cpool,
            tc.tile_pool(name="g", bufs=max(NPA, 1)) as gpool,
            tc.tile_pool(name="tA", bufs=2) as tapool,
            tc.tile_pool(name="outA", bufs=max(NPA, 1)) as oapool,
            tc.tile_pool(name="rhs", bufs=6) as rpool,
            tc.tile_pool(name="oh", bufs=6) as ohpool,
            tc.tile_pool(name="ps", bufs=2, space="PSUM") as pspool,
            tc.tile_pool(name="cc", bufs=3) as ccpool,
            tc.tile_pool(name="tB", bufs=3) as tbpool,
            tc.tile_pool(name="outB", bufs=6) as obpool,
        ):
            idx_all = cpool.tile([128, max(NPA, 1) * IDX_COLS], mybir.dt.int16)
            nc.sync.dma_start(out=idx_all[:], in_=gidx[:])

            deferred_stores = []

            def emit_a_pair(p):
                g = gpool.tile([A, PAIR * D, F], f16)
                nc.gpsimd.dma_gather(
                    out_ap=g[:],
                    in_ap=atoms16[:],
                    idxs_ap=idx_all[:, p * IDX_COLS:(p + 1) * IDX_COLS],
                    num_idxs=NI,
                    num_idxs_reg=NI,
                    elem_size=F,
                    single_packet=False,
                )
                slfa = rpool.tile([A, PAIR, F], f16, tag="slfa")
                nc.sync.dma_start(
                    out=slfa[:],
                    in_=selfm[p * PAIR * A:(p + 1) * PAIR * A, :].rearrange(
                        "(j p) f -> p j f", p=A))
                gv = g[:].rearrange("p (j s) f -> p j s f", s=D)
                t = tapool.tile([A, PAIR, 4, F], f16, tag="ta_t")
                nc.vector.tensor_tensor(
                    out=t[:], in0=gv[:, :, 0:8:2, :], in1=gv[:, :, 1:8:2, :], op=MAX)
                u = tapool.tile([A, PAIR, 2, F], f16, tag="ta_u")
                nc.vector.tensor_tensor(
                    out=u[:], in0=t[:, :, 0:2, :], in1=t[:, :, 2:4, :], op=MAX)
                v = tapool.tile([A, PAIR, F], f16, tag="ta_v")
                nc.vector.tensor_tensor(
                    out=v[:], in0=u[:, :, 0, :], in1=u[:, :, 1, :], op=MAX)
                w = oapool.tile([A, PAIR, F], f16, tag="wa")
                nc.vector.tensor_tensor(out=w[:], in0=v[:], in1=slfa[:], op=MAX)
                dst = out16[p * PAIR * A:(p + 1) * PAIR * A, :].rearrange(
                    "(j p) f -> p j f", p=A)
                deferred_stores.append((dst, w))

            def emit_b_mol(mi):
                m = NA + mi
                rhs = rpool.tile([A, F], f16, tag="rhs")
                nc.sync.dma_start(out=rhs[:], in_=atoms16[m * A:(m + 1) * A, :])
                oh = ohpool.tile([128, D, A], f16, tag="oh")
                nc.sync.dma_start(out=oh[:], in_=onehot[:, mi * D * A:(mi + 1) * D * A])
                slf = rpool.tile([A, F], f16, tag="slf")
                nc.sync.dma_start(out=slf[:], in_=selfm[m * A:(m + 1) * A, :])
                psa = pspool.tile([A, 4, F], f32, tag="ps")
                psb = pspool.tile([A, 4, F], f32, tag="ps")
                for d in range(4):
                    nc.tensor.matmul(psa[:, d, :], lhsT=oh[:, d, :], rhs=rhs[:],
                                     start=True, stop=True)
                for d in range(4):
                    nc.tensor.matmul(psb[:, d, :], lhsT=oh[:, 4 + d, :], rhs=rhs[:],
                                     start=True, stop=True)
                cc = ccpool.tile([A, 8, F], f16, tag="cc")
                nc.scalar.activation(
                    out=cc[:, 0:4, :], in_=psa[:],
                    func=mybir.ActivationFunctionType.Copy, bias=0.0, scale=1.0)
                nc.scalar.activation(
                    out=cc[:, 4:8, :], in_=psb[:],
                    func=mybir.ActivationFunctionType.Copy, bias=0.0, scale=1.0)
                t = tbpool.tile([A, 4, F], f16, tag="tb_t")
                nc.vector.tensor_tensor(
                    out=t[:], in0=cc[:, 0:8:2, :], in1=cc[:, 1:8:2, :], op=MAX)
                u = tbpool.tile([A, 2, F], f16, tag="tb_u")
                nc.vector.tensor_tensor(
                    out=u[:], in0=t[:, 0:2, :], in1=t[:, 2:4, :], op=MAX)
                v = tbpool.tile([A, F], f16, tag="tb_v")
                nc.vector.tensor_tensor(
                    out=v[:], in0=u[:, 0, :], in1=u[:, 1, :], op=MAX)
                o = obpool.tile([A, F], f16, tag="ob")
                nc.vector.tensor_tensor(out=o[:], in0=v[:], in1=slf[:], op=MAX)
                nc.sync.dma_start(out=out16[m * A:(m + 1) * A, :], in_=o[:])

            for p in range(max(NPA, len(b_chunks))):
                if p < len(b_chunks):
                    for mi in b_chunks[p]:
                        emit_b_mol(int(mi))
                if p < NPA:
                    emit_a_pair(p)
            for dst, w in deferred_stores:
                nc.sync.dma_start(out=dst, in_=w[:])
    nc.compile()
    _cached["nc"] = nc
    return nc


def _host_prep(atoms, bonds, edges):
    """Build per-core input maps. atoms (B,A,F) f32; edges (B,A,D) int32."""
    del bonds  # unused by the layer
    a_idx = np.arange(A, dtype=np.int64)[None, :, None]            # (1,A,1)
    e = edges.astype(np.int64)
    valid = e >= 0
    e_fixed = np.where(valid, e, a_idx)                            # (B,A,D)
    mask = valid.any(axis=2)                                       # (B,A) bool
    atoms16_full = atoms.astype(np.float16)                        # (B,A,F)

    in_maps = []
    for c in range(N_CORES):
        mol = slice(c * BPC, (c + 1) * BPC)
        at16 = np.zeros((TBL_ROWS, F), dtype=np.float16)
        at16[:BPC * A] = atoms16_full[mol].reshape(BPC * A, F)
        ef = e_fixed[mol]                                          # (BPC,A,D)
        msk = mask[mol]                                            # (BPC,A)

        # premasked self rows for all molecules
        sm = atoms16_full[mol] * msk[..., None]                    # (BPC,A,F) f16
        sm = np.ascontiguousarray(sm.reshape(BPC * A, F))

        # --- A path: dma_gather indices (pairs of molecules, 8 slots) ---
        base = (np.arange(BPC, dtype=np.int64) * A)[:, None, None]
        flat = ef + base                                           # (BPC,A,D)
        flat[~msk] = ZR                                            # masked -> zero row
        fa = flat[:NA].astype(np.int16)                            # (NA,A,D)
        per_pair = fa.transpose(0, 2, 1).reshape(max(NPA, 1), NI)  # i=(m%2)*D*A+s*A+a
        idx_lay = per_pair.reshape(max(NPA, 1), IDX_COLS, 16).transpose(0, 2, 1)
        idx16 = np.tile(idx_lay, (1, 8, 1)).transpose(1, 0, 2).reshape(
            128, max(NPA, 1) * IDX_COLS)
        idx16 = np.ascontiguousarray(idx16)

        # --- B path: fp16 one-hot [j, mi, d, a] ---
        efb = ef[NA:]                                              # (NB,A,D)
        mskb = msk[NA:]                                            # (NB,A)
        oh = np.zeros((NB, A, D, 128), dtype=np.float16)           # [mi,a,d,j]
        np.put_along_axis(oh, efb[..., None], 1.0, axis=3)
        oh[~mskb] = 0.0
        oh16 = oh.transpose(3, 0, 2, 1).reshape(128, NB * D * A)   # [j, mi*D*A+d*A+a]
        oh16 = np.ascontiguousarray(oh16)

        in_maps.append({"atoms16": at16, "gidx": idx16, "onehot": oh16, "selfm": sm})
    return in_maps


def kernel(atoms, bonds, edges, _want_timing=False, **_ignored):
    nc = _build_kernel()
    in_maps = _host_prep(np.asarray(atoms, dtype=np.float32), bonds,
                         np.asarray(edges, dtype=np.int32))
    res = run_bass_kernel_spmd(nc, in_maps, list(range(N_CORES)),
                               trace=bool(_want_timing))
    outs = [res.results[c]["out16"].reshape(BPC, A, F) for c in range(N_CORES)]
    full = np.concatenate(outs, axis=0).astype(np.float32)
    if _want_timing:
        return full, res
    return full


# revision 16
# speedup vs baseline: 2.1625x; 1.0133x over previous
"""NeuralGraphPool kernel for Trainium2 (8 NeuronCores, data-parallel over batch).

Computation (per molecule b):
    out[a, f] = max(atoms[a, f], max_{d: edges[a,d]>=0} atoms[edges[a,d], f])
                * (any edge valid ? 1 : 0)

Strategy (stage 4 — hybrid DMA-gather + TensorEngine one-hot gather):
  The SWDGE dma_gather path is limited by GPSIMD Q7 descriptor generation
  (~8.3 ns/row), so only NA molecules use it (8 neighbour slots only; self
  comes from a premasked table).  The other NB molecules are gathered on the
  TensorEngine: per (molecule, d) an fp16 one-hot matrix (host-built)
  selects neighbour rows via matmul into PSUM, ScalarE evacuates PSUM to
  fp16 SBUF (TT cannot read two PSUM operands), and the DVE max-tree
  reduces the planes + self.  Masking is folded into the inputs: masked
  atoms gather the zero row / all-zero one-hot columns, and the premasked
  self table has zeroed rows.  Output is fp16, cast to fp32 on the host.
  A-path stores are emitted last so they never head-of-line-block B loads
  in the Sync DMA stream.
"""

import numpy as np
import ml_dtypes

import concourse.bass as bass
import concourse.bacc as bacc
import concourse.mybir as mybir
from concourse import hw_specs
from concourse.tile import TileContext
from concourse.bass_utils import run_bass_kernel_spmd

# The stock cost model assumes SWDGE descriptor generation at 0.34 ns/desc
# (calibrated on bulk CounterMachine copies).  dma_gather's per-index ucode
# loop measures ~8.3 ns/desc on HW; with the stock value the Tile scheduler
# believes gathers are ~20x faster than reality and orders the gather-path
# reduction ops ahead of the tensor-path ones, serializing the kernel.
hw_specs.TRN2Spec.SWDGE_NS_PER_DESCRIPTOR = 8.3

# Problem constants (hardcoded per harness contract).
B, A, D, F = 256, 128, 8, 512
N_CORES = 8
BPC = B // N_CORES           # molecules per core (32)
PAIR = 2                     # molecules per gather batch (A-path)
NA = 10                      # molecules on the dma_gather path (must be even)
NB = BPC - NA                # molecules on the tensor-engine path
NPA = NA // PAIR             # gather pairs
NI = PAIR * D * A            # gather indices per pair (2048; neighbours only)
IDX_COLS = NI // 16          # idx free-dim per pair (128)
ZR = BPC * A                 # zero-row index in the atoms16 table
TBL_ROWS = BPC * A + 128     # table rows (last 128 are zeros)

_cached = {}


def _build_kernel():
    if "nc" in _cached:
        return _cached["nc"]
    nc = bacc.Bacc("TRN2", num_devices=N_CORES)
    f16 = mybir.dt.float16
    f32 = mybir.dt.float32
    MAX = mybir.AluOpType.max
    atoms16 = nc.declare_dram_parameter("atoms16", [TBL_ROWS, F], f16, isOutput=False)
    gidx = nc.declare_dram_parameter("gidx", [128, max(NPA, 1) * IDX_COLS],
                                     mybir.dt.int16, isOutput=False)
    onehot = nc.declare_dram_parameter("onehot", [128, max(NB, 1) * D * A], f16,
                                       isOutput=False)
    selfm = nc.declare_dram_parameter("selfm", [BPC * A, F], f16, isOutput=False)
    out16 = nc.declare_dram_parameter("out16", [BPC * A, F], f16, isOutput=True)

    # Front-load B work: gather p completes at ~17*(p+1) us, so the DVE must
    # chew ~8 B molecules before the first A tree and ~4 between later ones.
    first = max(1, (2 * NB) // 5)
    rest = np.array_split(np.arange(first, NB), max(NPA - 1, 1))
    b_chunks = [np.arange(first)] + list(rest) if NPA > 1 else [np.arange(NB)]

    with TileContext(nc) as tc:
        with (
            tc.tile_pool(name="const", bufs=1) as cpool,
            tc.tile_pool(name="g", bufs=max(NPA, 1)) as gpool,
            tc.tile_pool(name="tA", bufs=2) as tapool,
            tc.tile_pool(name="outA", bufs=max(NPA, 1)) as oapool,
            tc.tile_pool(name="rhs", bufs=6) as rpool,
            tc.tile_pool(name="oh", bufs=6) as ohpool,
            tc.tile_pool(name="ps", bufs=2, space="PSUM") as pspool,
            tc.tile_pool(name="cc", bufs=3) as ccpool,
            tc.tile_pool(name="tB", bufs=3) as tbpool,
            tc.tile_pool(name="outB", bufs=6) as obpool,
        ):
            idx_all = cpool.tile([128, max(NPA, 1) * IDX_COLS], mybir.dt.int16)
            nc.sync.dma_start(out=idx_all[:], in_=gidx[:])

            deferred_stores = []

            def emit_a_pair(p):
                g = gpool.tile([A, PAIR * D, F], f16)
                nc.gpsimd.dma_gather(
                    out_ap=g[:],
                    in_ap=atoms16[:],
                    idxs_ap=idx_all[:, p * IDX_COLS:(p + 1) * IDX_COLS],
                    num_idxs=NI,
                    num_idxs_reg=NI,
                    elem_size=F,
                    single_packet=False,
                )
                slfa = rpool.tile([A, PAIR, F], f16, tag="slfa")
                nc.sync.dma_start(
                    out=slfa[:],
                    in_=selfm[p * PAIR * A:(p + 1) * PAIR * A, :].rearrange(
                        "(j p) f -> p j f", p=A))
                gv = g[:].rearrange("p (j s) f -> p j s f", s=D)
                t = tapool.tile([A, PAIR, 4, F], f16, tag="ta_t")
                nc.vector.tensor_tensor(
                    out=t[:], in0=gv[:, :, 0:8:2, :], in1=gv[:, :, 1:8:2, :], op=MAX)
                u = tapool.tile([A, PAIR, 2, F], f16, tag="ta_u")
                nc.vector.tensor_tensor(
                    out=u[:], in0=t[:, :, 0:2, :], in1=t[:, :, 2:4, :], op=MAX)
                v = tapool.tile([A, PAIR, F], f16, tag="ta_v")
                nc.vector.tensor_tensor(
                    out=v[:], in0=u[:, :, 0, :], in1=u[:, :, 1, :], op=MAX)
                w = oapool.tile([A, PAIR, F], f16, tag="wa")
                nc.vector.tensor_tensor(out=w[:], in0=v[:], in1=slfa[:], op=MAX)
                dst = out16[p * PAIR * A:(p + 1) * PAIR * A, :].rearrange(
                    "(j p) f -> p j f", p=A)
                deferred_stores.append((dst, w))

            def emit_b_mol(mi):
                m = NA + mi
                rhs = rpool.tile([A, F], f16, tag="rhs")
                nc.sync.dma_start(out=rhs[:], in_=atoms16[m * A:(m + 1) * A, :])
                oh = ohpool.tile([128, D, A], f16, tag="oh")
                nc.sync.dma_start(out=oh[:], in_=onehot[:, mi * D * A:(mi + 1) * D * A])
                slf = rpool.tile([A, F], f16, tag="slf")
                nc.sync.dma_start(out=slf[:], in_=selfm[m * A:(m + 1) * A, :])
                psa = pspool.tile([A, 4, F], f32, tag="ps")
                psb = pspool.tile([A, 4, F], f32, tag="ps")
                for d in range(4):
                    nc.tensor.matmul(psa[:, d, :], lhsT=oh[:, d, :], rhs=rhs[:],
                                     start=True, stop=True)
                for d in range(4):
                    nc.tensor.matmul(psb[:, d, :], lhsT=oh[:, 4 + d, :], rhs=rhs[:],
                                     start=True, stop=True)
                cc = ccpool.tile([A, 8, F], f16, tag="cc")
                nc.scalar.activation(
                    out=cc[:, 0:4, :], in_=psa[:],
                    func=mybir.ActivationFunctionType.Copy, bias=0.0, scale=1.0)
                nc.scalar.activation(
                    out=cc[:, 4:8, :], in_=psb[:],
                    func=mybir.ActivationFunctionType.Copy, bias=0.0, scale=1.0)
                t = tbpool.tile([A, 4, F], f16, tag="tb_t")
                nc.vector.tensor_tensor(
                    out=t[:], in0=cc[:, 0:8:2, :], in1=cc[:, 1:8:2, :], op=MAX)
                u = tbpool.tile([A, 2, F], f16, tag="tb_u")
                nc.vector.tensor_tensor(
                    out=u[:], in0=t[:, 0:2, :], in1=t[:, 2:4, :], op=MAX)
                v = tbpool.tile([A, F], f16, tag="tb_v")
                nc.vector.tensor_tensor(
                    out=v[:], in0=u[:, 0, :], in1=u[:, 1, :], op=MAX)
                o = obpool.tile([A, F], f16, tag="ob")
                nc.vector.tensor_tensor(out=o[:], in0=v[:], in1=slf[:], op=MAX)
                nc.sync.dma_start(out=out16[m * A:(m + 1) * A, :], in_=o[:])

            for p in range(max(NPA, len(b_chunks))):
                if p < len(b_chunks):
                    for mi in b_chunks[p]:
                        emit_b_mol(int(mi))
                if p < NPA:
                    emit_a_pair(p)
            for dst, w in deferred_stores:
                nc.sync.dma_start(out=dst, in_=w[:])
    nc.compile()
    _cached["nc"] = nc
    return nc


def _host_prep(atoms, bonds, edges):
    """Build per-core input maps. atoms (B,A,F) f32; edges (B,A,D) int32."""
    del bonds  # unused by the layer
    a_idx = np.arange(A, dtype=np.int64)[None, :, None]            # (1,A,1)
    e = edges.astype(np.int64)
    valid = e >= 0
    e_fixed = np.where(valid, e, a_idx)                            # (B,A,D)
    mask = valid.any(axis=2)                                       # (B,A) bool
    atoms16_full = atoms.astype(np.float16)                        # (B,A,F)

    in_maps = []
    for c in range(N_CORES):
        mol = slice(c * BPC, (c + 1) * BPC)
        at16 = np.zeros((TBL_ROWS, F), dtype=np.float16)
        at16[:BPC * A] = atoms16_full[mol].reshape(BPC * A, F)
        ef = e_fixed[mol]                                          # (BPC,A,D)
        msk = mask[mol]                                            # (BPC,A)

        # premasked self rows for all molecules
        sm = atoms16_full[mol] * msk[..., None]                    # (BPC,A,F) f16
        sm = np.ascontiguousarray(sm.reshape(BPC * A, F))

        # --- A path: dma_gather indices (pairs of molecules, 8 slots) ---
        base = (np.arange(BPC, dtype=np.int64) * A)[:, None, None]
        flat = ef + base                                           # (BPC,A,D)
        flat[~msk] = ZR                                            # masked -> zero row
        fa = flat[:NA].astype(np.int16)                            # (NA,A,D)
        per_pair = fa.transpose(0, 2, 1).reshape(max(NPA, 1), NI)  # i=(m%2)*D*A+s*A+a
        idx_lay = per_pair.reshape(max(NPA, 1), IDX_COLS, 16).transpose(0, 2, 1)
        idx16 = np.tile(idx_lay, (1, 8, 1)).transpose(1, 0, 2).reshape(
            128, max(NPA, 1) * IDX_COLS)
        idx16 = np.ascontiguousarray(idx16)

        # --- B path: fp16 one-hot [j, mi, d, a] ---
        efb = ef[NA:]                                              # (NB,A,D)
        mskb = msk[NA:]                                            # (NB,A)
        oh = np.zeros((NB, A, D, 128), dtype=np.float16)           # [mi,a,d,j]
        np.put_along_axis(oh, efb[..., None], 1.0, axis=3)
        oh[~mskb] = 0.0
        oh16 = oh.transpose(3, 0, 2, 1).reshape(128, NB * D * A)   # [j, mi*D*A+d*A+a]
        oh16 = np.ascontiguousarray(oh16)

        in_maps.append({"atoms16": at16, "gidx": idx16, "onehot": oh16, "selfm": sm})
    return in_maps


def kernel(atoms, bonds, edges, _want_timing=False, **_ignored):
    nc = _build_kernel()
    in_maps = _host_prep(np.asarray(atoms, dtype=np.float32), bonds,
                         np.asarray(edges, dtype=np.int32))
    res = run_bass_kernel_spmd(nc, in_maps, list(range(N_CORES)),
                               trace=bool(_want_timing))
    outs = [res.results[c]["out16"].reshape(BPC, A, F) for c in range(N_CORES)]
    full = np.concatenate(outs, axis=0).astype(np.float32)
    if _want_timing:
        return full, res
    return full


# revision 17
# speedup vs baseline: 2.6435x; 1.2224x over previous
"""NeuralGraphPool kernel for Trainium2 (8 NeuronCores, data-parallel over batch).

Computation (per molecule b):
    out[a, f] = max(atoms[a, f], max_{d: edges[a,d]>=0} atoms[edges[a,d], f])
                * (any edge valid ? 1 : 0)

Strategy (stage 4 — hybrid DMA-gather + TensorEngine one-hot gather):
  The SWDGE dma_gather path is limited by GPSIMD Q7 descriptor generation
  (~8.3 ns/row), so only NA molecules use it (8 neighbour slots only; self
  comes from a premasked table).  The other NB molecules are gathered on the
  TensorEngine: per (molecule, d) an fp16 one-hot matrix (host-built)
  selects neighbour rows via matmul into PSUM, ScalarE evacuates PSUM to
  fp16 SBUF (TT cannot read two PSUM operands), and the DVE max-tree
  reduces the planes + self.  Masking is folded into the inputs: masked
  atoms gather the zero row / all-zero one-hot columns, and the premasked
  self table has zeroed rows.  Output is fp16, cast to fp32 on the host.
  A-path stores are emitted last so they never head-of-line-block B loads
  in the Sync DMA stream.
"""

import numpy as np
import ml_dtypes

import concourse.bass as bass
import concourse.bacc as bacc
import concourse.mybir as mybir
from concourse import hw_specs
from concourse.tile import TileContext
from concourse.bass_utils import run_bass_kernel_spmd

# The stock cost model misprices dma_gather: the scheduler's (v1) model has
# no InstDMAGatherAnt visitor, so a gather falls to visit_default and is
# priced at ~7 us when the per-index Q7 ucode loop really takes ~17-20 us.
# The Tile scheduler therefore orders the gather-path reduction ops ahead of
# the tensor-path ones and the gather head-of-line-blocks the DVE stream for
# ~40 us.  Only the gather runs on the Pool engine in this kernel, so
# inflating the Pool cycle time makes the simulated schedule match HW
# reality.  (SWDGE_NS_PER_DESCRIPTOR feeds the v2 model only, patched for
# consistency.)
hw_specs.TRN2Spec.SWDGE_NS_PER_DESCRIPTOR = 8.3
hw_specs.TRN2Spec.CYCLE_T[mybir.EngineType.Pool] = 3.5 * (1e9 / 1.2e9)

# Problem constants (hardcoded per harness contract).
B, A, D, F = 256, 128, 8, 512
N_CORES = 8
BPC = B // N_CORES           # molecules per core (32)
PAIR = 2                     # molecules per gather batch (A-path)
NA = 10                      # molecules on the dma_gather path (must be even)
NB = BPC - NA                # molecules on the tensor-engine path
NPA = NA // PAIR             # gather pairs
NI = PAIR * D * A            # gather indices per pair (2048; neighbours only)
IDX_COLS = NI // 16          # idx free-dim per pair (128)
ZR = BPC * A                 # zero-row index in the atoms16 table
TBL_ROWS = BPC * A + 128     # table rows (last 128 are zeros)

_cached = {}


def _build_kernel():
    if "nc" in _cached:
        return _cached["nc"]
    nc = bacc.Bacc("TRN2", num_devices=N_CORES)
    f16 = mybir.dt.float16
    f32 = mybir.dt.float32
    MAX = mybir.AluOpType.max
    atoms16 = nc.declare_dram_parameter("atoms16", [TBL_ROWS, F], f16, isOutput=False)
    gidx = nc.declare_dram_parameter("gidx", [128, max(NPA, 1) * IDX_COLS],
                                     mybir.dt.int16, isOutput=False)
    onehot = nc.declare_dram_parameter("onehot", [128, max(NB, 1) * D * A], f16,
                                       isOutput=False)
    selfm = nc.declare_dram_parameter("selfm", [BPC * A, F], f16, isOutput=False)
    out16 = nc.declare_dram_parameter("out16", [BPC * A, F], f16, isOutput=True)

    # Front-load B work: gather p completes at ~17*(p+1) us, so the DVE must
    # chew ~8 B molecules before the first A tree and ~4 between later ones.
    first = max(1, (2 * NB) // 5)
    rest = np.array_split(np.arange(first, NB), max(NPA - 1, 1))
    b_chunks = [np.arange(first)] + list(rest) if NPA > 1 else [np.arange(NB)]

    with TileContext(nc) as tc:
        with (
            tc.tile_pool(name="const", bufs=1) as cpool,
            tc.tile_pool(name="g", bufs=max(NPA, 1)) as gpool,
            tc.tile_pool(name="tA", bufs=2) as tapool,
            tc.tile_pool(name="outA", bufs=max(NPA, 1)) as oapool,
            tc.tile_pool(name="rhs", bufs=6) as rpool,
            tc.tile_pool(name="oh", bufs=6) as ohpool,
            tc.tile_pool(name="ps", bufs=2, space="PSUM") as pspool,
            tc.tile_pool(name="cc", bufs=3) as ccpool,
            tc.tile_pool(name="tB", bufs=3) as tbpool,
            tc.tile_pool(name="outB", bufs=6) as obpool,
        ):
            idx_all = cpool.tile([128, max(NPA, 1) * IDX_COLS], mybir.dt.int16)
            nc.sync.dma_start(out=idx_all[:], in_=gidx[:])

            deferred_stores = []

            def emit_a_pair(p):
                g = gpool.tile([A, PAIR * D, F], f16)
                nc.gpsimd.dma_gather(
                    out_ap=g[:],
                    in_ap=atoms16[:],
                    idxs_ap=idx_all[:, p * IDX_COLS:(p + 1) * IDX_COLS],
                    num_idxs=NI,
                    num_idxs_reg=NI,
                    elem_size=F,
                    single_packet=False,
                )
                slfa = rpool.tile([A, PAIR, F], f16, tag="slfa")
                nc.sync.dma_start(
                    out=slfa[:],
                    in_=selfm[p * PAIR * A:(p + 1) * PAIR * A, :].rearrange(
                        "(j p) f -> p j f", p=A))
                gv = g[:].rearrange("p (j s) f -> p j s f", s=D)
                t = tapool.tile([A, PAIR, 4, F], f16, tag="ta_t")
                nc.vector.tensor_tensor(
                    out=t[:], in0=gv[:, :, 0:8:2, :], in1=gv[:, :, 1:8:2, :], op=MAX)
                u = tapool.tile([A, PAIR, 2, F], f16, tag="ta_u")
                nc.vector.tensor_tensor(
                    out=u[:], in0=t[:, :, 0:2, :], in1=t[:, :, 2:4, :], op=MAX)
                v = tapool.tile([A, PAIR, F], f16, tag="ta_v")
                nc.vector.tensor_tensor(
                    out=v[:], in0=u[:, :, 0, :], in1=u[:, :, 1, :], op=MAX)
                w = oapool.tile([A, PAIR, F], f16, tag="wa")
                nc.vector.tensor_tensor(out=w[:], in0=v[:], in1=slfa[:], op=MAX)
                dst = out16[p * PAIR * A:(p + 1) * PAIR * A, :].rearrange(
                    "(j p) f -> p j f", p=A)
                deferred_stores.append((dst, w))

            def emit_b_mol(mi):
                m = NA + mi
                rhs = rpool.tile([A, F], f16, tag="rhs")
                nc.sync.dma_start(out=rhs[:], in_=atoms16[m * A:(m + 1) * A, :])
                oh = ohpool.tile([128, D, A], f16, tag="oh")
                nc.sync.dma_start(out=oh[:], in_=onehot[:, mi * D * A:(mi + 1) * D * A])
                slf = rpool.tile([A, F], f16, tag="slf")
                nc.sync.dma_start(out=slf[:], in_=selfm[m * A:(m + 1) * A, :])
                psa = pspool.tile([A, 4, F], f32, tag="ps")
                psb = pspool.tile([A, 4, F], f32, tag="ps")
                for d in range(4):
                    nc.tensor.matmul(psa[:, d, :], lhsT=oh[:, d, :], rhs=rhs[:],
                                     start=True, stop=True)
                for d in range(4):
                    nc.tensor.matmul(psb[:, d, :], lhsT=oh[:, 4 + d, :], rhs=rhs[:],
                                     start=True, stop=True)
                cc = ccpool.tile([A, 8, F], f16, tag="cc")
                nc.scalar.activation(
                    out=cc[:, 0:4, :], in_=psa[:],
                    func=mybir.ActivationFunctionType.Copy, bias=0.0, scale=1.0)
                nc.scalar.activation(
                    out=cc[:, 4:8, :], in_=psb[:],
                    func=mybir.ActivationFunctionType.Copy, bias=0.0, scale=1.0)
                t = tbpool.tile([A, 4, F], f16, tag="tb_t")
                nc.vector.tensor_tensor(
                    out=t[:], in0=cc[:, 0:8:2, :], in1=cc[:, 1:8:2, :], op=MAX)
                u = tbpool.tile([A, 2, F], f16, tag="tb_u")
                nc.vector.tensor_tensor(
                    out=u[:], in0=t[:, 0:2, :], in1=t[:, 2:4, :], op=MAX)
                v = tbpool.tile([A, F], f16, tag="tb_v")
                nc.vector.tensor_tensor(
                    out=v[:], in0=u[:, 0, :], in1=u[:, 1, :], op=MAX)
                o = obpool.tile([A, F], f16, tag="ob")
                nc.vector.tensor_tensor(out=o[:], in0=v[:], in1=slf[:], op=MAX)
                nc.sync.dma_start(out=out16[m * A:(m + 1) * A, :], in_=o[:])

            for p in range(max(NPA, len(b_chunks))):
                if p < len(b_chunks):
                    for mi in b_chunks[p]:
                        emit_b_mol(int(mi))
                if p < NPA:
                    emit_a_pair(p)
            for dst, w in deferred_stores:
                nc.sync.dma_start(out=dst, in_=w[:])
    nc.compile()
    _cached["nc"] = nc
    return nc


def _host_prep(atoms, bonds, edges):
    """Build per-core input maps. atoms (B,A,F) f32; edges (B,A,D) int32."""
    del bonds  # unused by the layer
    a_idx = np.arange(A, dtype=np.int64)[None, :, None]            # (1,A,1)
    e = edges.astype(np.int64)
    valid = e >= 0
    e_fixed = np.where(valid, e, a_idx)                            # (B,A,D)
    mask = valid.any(axis=2)                                       # (B,A) bool
    atoms16_full = atoms.astype(np.float16)                        # (B,A,F)

    in_maps = []
    for c in range(N_CORES):
        mol = slice(c * BPC, (c + 1) * BPC)
        at16 = np.zeros((TBL_ROWS, F), dtype=np.float16)
        at16[:BPC * A] = atoms16_full[mol].reshape(BPC * A, F)
        ef = e_fixed[mol]                                          # (BPC,A,D)
        msk = mask[mol]                                            # (BPC,A)

        # premasked self rows for all molecules
        sm = atoms16_full[mol] * msk[..., None]                    # (BPC,A,F) f16
        sm = np.ascontiguousarray(sm.reshape(BPC * A, F))

        # --- A path: dma_gather indices (pairs of molecules, 8 slots) ---
        base = (np.arange(BPC, dtype=np.int64) * A)[:, None, None]
        flat = ef + base                                           # (BPC,A,D)
        flat[~msk] = ZR                                            # masked -> zero row
        fa = flat[:NA].astype(np.int16)                            # (NA,A,D)
        per_pair = fa.transpose(0, 2, 1).reshape(max(NPA, 1), NI)  # i=(m%2)*D*A+s*A+a
        idx_lay = per_pair.reshape(max(NPA, 1), IDX_COLS, 16).transpose(0, 2, 1)
        idx16 = np.tile(idx_lay, (1, 8, 1)).transpose(1, 0, 2).reshape(
            128, max(NPA, 1) * IDX_COLS)
        idx16 = np.ascontiguousarray(idx16)

        # --- B path: fp16 one-hot [j, mi, d, a] ---
        efb = ef[NA:]                                              # (NB,A,D)
        mskb = msk[NA:]                                            # (NB,A)
        oh = np.zeros((NB, A, D, 128), dtype=np.float16)           # [mi,a,d,j]
        np.put_along_axis(oh, efb[..., None], 1.0, axis=3)
        oh[~mskb] = 0.0
        oh16 = oh.transpose(3, 0, 2, 1).reshape(128, NB * D * A)   # [j, mi*D*A+d*A+a]
        oh16 = np.ascontiguousarray(oh16)

        in_maps.append({"atoms16": at16, "gidx": idx16, "onehot": oh16, "selfm": sm})
    return in_maps


def kernel(atoms, bonds, edges, _want_timing=False, **_ignored):
    nc = _build_kernel()
    in_maps = _host_prep(np.asarray(atoms, dtype=np.float32), bonds,
                         np.asarray(edges, dtype=np.int32))
    res = run_bass_kernel_spmd(nc, in_maps, list(range(N_CORES)),
                               trace=bool(_want_timing))
    outs = [res.results[c]["out16"].reshape(BPC, A, F) for c in range(N_CORES)]
    full = np.concatenate(outs, axis=0).astype(np.float32)
    if _want_timing:
        return full, res
    return full
